# revision 1
# baseline (speedup 1.0000x reference)
"""BinaryTreeRNN Trainium2 kernel — 8-core data-parallel.

Contract: kernel(**inputs) takes FULL unsharded inputs (x [4M,16] f32 plus tiny
tree params) and returns the FULL [4M] f32 output.

Design (per core, N_core = 500k samples, padded to 501760 = 560 blocks x 896):
  * Host folds all tree parameters:  softmax(om) -> per-node (A, P, R, phi, B);
    S*sin(s)+C*cos(s) == R*sin(s+phi).  Level-(l-1) phases are pushed into the
    children's outputs (out' = out + phi_parent/2) with exact algebraic
    compensation in the combine coefficients.
  * Host packs x into per-block stationary tiles xt[blk] = [128, 128]:
    rows 16a+v (a<7) hold x[blk*896 + 7p + a, v] in column p; rows 112..127
    hold 1.0 (constant slot -> every matmul column gets a free additive bias).
  * PE: per block, one fp32 matmul  out[p, c] = sum_k xt[k, p] * Wpat[k, c]
    producing 12 functions x 7 sample-slots = 84 columns, sample-major:
    cols [0,28) = biased left leaves (nodes 0..3), [28,56) biased right
    leaves, [56,84) = s3C = l+r+phi3 (sin-ready sums).
  * DVE/ACT tree: products via tensor_mul; range reduction via the
    round-to-nearest MAGIC trick  sc = s/2pi; k' = sc+MAGIC; f = (k'-MAGIC)-sc;
    sin(s+phi) = Sin(f * -2pi) on the scalar engine; combine via
    ln_bwd_dx (R*t + A*s + beta in one op) + affine_then_add (+P*p).
"""

import os
import sys

for _p in ("/opt/trn_rl_repo", "/root/.axon_site/_ro/trn_rl_repo"):
    if os.path.isdir(_p) and _p not in sys.path:
        sys.path.append(_p)

import numpy as np

N_FULL = 4_000_000
V = 16
N_CORES = 8
N_CORE = N_FULL // N_CORES          # 500_000
SLOTS = 7                            # samples per stationary column
BLK = 128 * SLOTS                    # 896 samples per matmul block
N_BLOCKS = 560                       # ceil(500000/896) -> padded
N_PAD = N_BLOCKS * BLK               # 501_760
B = 16                               # blocks per super-tile
N_ST = N_BLOCKS // B                 # 35

MAGIC = float(np.float32(1.5 * 2**23))
INV2PI = float(np.float32(1.0 / (2.0 * np.pi)))
NEG2PI = float(np.float32(-2.0 * np.pi))

F32 = np.float32


def _softmax64(om):
    e = np.exp(om.astype(np.float64) - om.astype(np.float64).max(-1, keepdims=True))
    return e / e.sum(-1, keepdims=True)


def _fold(leaf_w, leaf_b, w1, b1, om1, w2, b2, om2, w3, b3, om3):
    """float64 constant folding. Returns per-level dicts + matmul pattern."""
    lv = {}
    for lvl, (w, b, om) in {3: (w3, b3, om3), 2: (w2, b2, om2), 1: (w1, b1, om1)}.items():
        sm = _softmax64(om)
        w64 = w.astype(np.float64)
        A = w64 * sm[:, 0]
        S = w64 * sm[:, 1]
        C = w64 * sm[:, 2]
        P = w64 * sm[:, 3]
        R = np.hypot(S, C)
        phi = np.arctan2(C, S)
        lv[lvl] = dict(A=A, B=b.astype(np.float64), P=P, R=R, phi=phi)

    # combine: out = Aeff*sC + R*t + P*p + beta, via
    #   u0 = p*P + beta (TS); u1 = t*R + u0 (STT); out = sC*Aeff + u1 (STT)
    def cparams(Aeff, beta, R, P):
        return dict(A=float(F32(Aeff)), beta=float(F32(beta)),
                    R=float(F32(R)), P=float(F32(P)))

    c3, c2, c1 = lv[3], lv[2], lv[1]
    L3 = []
    for n in range(4):
        delta = c2["phi"][n // 2] / 2.0
        beta = c3["B"][n] - c3["A"][n] * c3["phi"][n] + delta
        L3.append(cparams(c3["A"][n], beta, c3["R"][n], c3["P"][n]))
    L2 = []
    for m in range(2):
        ph = c2["phi"][m]
        Aeff = c2["A"][m] - c2["P"][m] * ph / 2.0
        delta = c1["phi"][0] / 2.0
        beta = c2["B"][m] - c2["A"][m] * ph + c2["P"][m] * ph * ph / 4.0 + delta
        L2.append(cparams(Aeff, beta, c2["R"][m], c2["P"][m]))
    ph = c1["phi"][0]
    Aeff = c1["A"][0] - c1["P"][0] * ph / 2.0
    beta = c1["B"][0] - c1["A"][0] * ph + c1["P"][0] * ph * ph / 4.0
    L1 = [cparams(Aeff, beta, c1["R"][0], c1["P"][0])]

    # Wpat [128, 84]: col 7j+a, j = 0..11 functions, a = 0..6 slots.
    # rows 16a+v: weight of x[., v] for slot a;  rows 112+v: constant (only v=0 used).
    wp = np.zeros((128, 84), np.float64)
    lw = leaf_w.astype(np.float64)
    lb = leaf_b.astype(np.float64)
    for n in range(4):
        funcs = [
            (n, lw[2 * n], lb[2 * n]),                               # hl'
            (4 + n, lw[2 * n + 1], lb[2 * n + 1]),                   # hr'
            (8 + n, lw[2 * n] + lw[2 * n + 1],
             lb[2 * n] + lb[2 * n + 1] + c3["phi"][n]),              # s3C
        ]
        for j, wv, bias in funcs:
            for a in range(SLOTS):
                wp[16 * a: 16 * a + 16, 7 * j + a] = wv
                wp[112, 7 * j + a] = bias
    wp32 = wp  # float64
    wph = wp32.astype(np.float16)
    wpl = (wp32 - wph.astype(np.float64)).astype(np.float16)
    wp2 = np.concatenate([wph, wpl], axis=1)       # [128, 168] fp16
    return L3, L2, L1, wp2


def _pack_x(x_shard, n_st=N_ST, b_blocks=B):
    """[n, 16] f32 -> (xt_hi, xt_lo) fp16 [n_st, 128, b_blocks*128]:
    split-fp16 stationary tiles, per-partition supertile-contiguous lines."""
    npad = n_st * b_blocks * BLK
    xs = np.empty((npad, V), F32)
    xs[:len(x_shard)] = x_shard
    xs[len(x_shard):] = 1.0
    a = xs.reshape(n_st, b_blocks, 128, SLOTS, V)  # [st, b, p, a, v]
    xt = np.empty((n_st, 128, b_blocks, 128), F32)
    xt[:, :112] = a.transpose(0, 3, 4, 1, 2).reshape(n_st, 112, b_blocks, 128)
    xt[:, 112:] = 1.0
    xt = xt.reshape(n_st, 128, b_blocks * 128)
    xh = xt.astype(np.float16)
    xl = (xt - xh.astype(F32)).astype(np.float16)
    return xh, xl


_PROGRAM_CACHE = {}


def _build_program(n_st=N_ST, b_blocks=B):
    """Build + compile the per-core Bass program (identical on all cores)."""
    import json
    key = (n_st, b_blocks, json.dumps(_build_program.consts, sort_keys=True, default=str))
    if key in _PROGRAM_CACHE:
        return _PROGRAM_CACHE[key]

    import concourse.bass as bass
    import concourse.tile as tile
    from concourse import bacc, mybir
    from contextlib import ExitStack

    f32 = mybir.dt.float32
    Sin = mybir.ActivationFunctionType.Sin
    sub = mybir.AluOpType.subtract
    mult = mybir.AluOpType.mult
    addop = mybir.AluOpType.add
    nb = n_st * b_blocks

    nc = bacc.Bacc("TRN2", target_bir_lowering=False, debug=False,
                   num_devices=N_CORES)
    f16 = mybir.dt.float16
    xh_d = nc.dram_tensor("xh", [n_st, 128, b_blocks * 128], f16,
                          kind="ExternalInput")
    xl_d = nc.dram_tensor("xl", [n_st, 128, b_blocks * 128], f16,
                          kind="ExternalInput")
    wp_d = nc.dram_tensor("wp", [128, 168], f16, kind="ExternalInput")
    out_d = nc.dram_tensor("out", [n_st, 128, b_blocks, SLOTS], f32,
                           kind="ExternalOutput")

    # Constants are baked as immediates; read them from the module-level holder.
    L3, L2, L1 = _build_program.consts
    Ident = mybir.ActivationFunctionType.Identity
    GROUP = 5

    # Activation float biases require pre-registered const APs.
    def reg_const(v):
        if (f32, v) not in nc.const_aps.aps:
            t = nc.alloc_sbuf_tensor(
                f"constx-{len(nc.const_aps.aps)}", [128, 1], f32)
            nc.gpsimd.memset(t.ap(), v)
            nc.const_aps.aps[(f32, v)] = t.ap()

    for cn in L3 + L2 + L1:
        reg_const(cn["beta"])
    nc.all_engine_barrier()

    with tile.TileContext(nc) as tc:
        with ExitStack() as ctx:
            const_pool = ctx.enter_context(tc.tile_pool(name="const", bufs=1))
            xpool = ctx.enter_context(tc.tile_pool(name="x", bufs=2))
            ppool = ctx.enter_context(
                tc.tile_pool(name="ps", bufs=2, space=bass.MemorySpace.PSUM))
            wpool = ctx.enter_context(tc.tile_pool(name="w", bufs=2))
            gpool = ctx.enter_context(tc.tile_pool(name="g", bufs=2))

            wp = const_pool.tile([128, 168], f16)
            nc.sync.dma_start(wp[:], wp_d[:])

            def tt(pool, cols, nm):
                t = pool.tile([128, cols], f32, name=nm, tag=nm)
                return t, t[:].rearrange("p (b c) -> p b c", c=ccols[nm])

            st0 = 0
            while st0 < n_st:
                glen = min(GROUP, n_st - st0)
                q = glen * b_blocks
                ccols = {"hrc": 28, "s3Cg": 28, "p3g": 28, "sc3g": 28,
                         "k3g": 28, "f3g": 28, "t3g": 28, "u0g": 28,
                         "u1g": 28, "o3acc": 28,
                         "s2": 14, "p2": 14, "sc2": 14, "k2": 14,
                         "f2": 14, "t2": 14, "u0_2": 14, "u1_2": 14, "o2": 14,
                         "s1": 7, "p1": 7, "sc1": 7, "k1": 7, "f1": 7,
                         "t1": 7, "u0_1": 7, "u1_1": 7, "yo": 7}

                def gt(nm, bufs=1):
                    c = ccols[nm]
                    t = gpool.tile([128, GROUP * b_blocks * c], f32,
                                   name=nm, tag=nm, bufs=bufs)
                    return t, t[:].rearrange("p (q c) -> p q c", c=c)

                s3Cg, s3Cgv = gt("s3Cg", bufs=2)
                p3g, p3gv = gt("p3g", bufs=2)
                o3acc, o3accv = gt("o3acc")

                for seg in range(glen):
                    st = st0 + seg
                    x2h = xpool.tile([128, b_blocks * 128], f16, name="x2h",
                                     tag="x2h")
                    nc.sync.dma_start(x2h[:], xh_d[st])
                    x2l = xpool.tile([128, b_blocks * 128], f16, name="x2l",
                                     tag="x2l")
                    nc.sync.dma_start(x2l[:], xl_d[st])

                    ps = ppool.tile([128, b_blocks * 128], f32)
                    for b in range(b_blocks):
                        o = ps[:, 128 * b:128 * b + 84]
                        xhb = x2h[:, 128 * b:128 * b + 128]
                        xlb = x2l[:, 128 * b:128 * b + 128]
                        nc.tensor.matmul(o, xhb, wp[:, 0:84],
                                         start=True, stop=False)
                        nc.tensor.matmul(o, xhb, wp[:, 84:168],
                                         start=False, stop=False)
                        nc.tensor.matmul(o, xlb, wp[:, 0:84],
                                         start=False, stop=True)
                    psv = ps[:].rearrange("p (b c) -> p b c", c=128)
                    segsl = slice(seg * b_blocks, (seg + 1) * b_blocks)

                    # stage hr (per-st) + s3C (group buffer) in SBUF via ACT
                    hrc = wpool.tile([128, b_blocks * 28], f32, name="hrc",
                                     tag="hrc")
                    hrcv = hrc[:].rearrange("p (b c) -> p b c", c=28)
                    nc.scalar.copy(hrcv, psv[:, :, 28:56])
                    nc.scalar.copy(s3Cgv[:, segsl, :], psv[:, :, 56:84])
                    nc.vector.tensor_mul(p3gv[:, segsl, :],
                                         psv[:, :, 0:28], hrcv)

                # ---- level 3 (batched over the group) ----
                qf28 = q * 28
                sc3g, _ = gt("sc3g")
                nc.vector.tensor_scalar_mul(sc3g[:, 0:qf28], s3Cg[:, 0:qf28],
                                            INV2PI)
                k3g, _ = gt("k3g")
                nc.vector.tensor_scalar_add(k3g[:, 0:qf28], sc3g[:, 0:qf28],
                                            MAGIC)
                f3g, _ = gt("f3g")
                nc.vector.scalar_tensor_tensor(f3g[:, 0:qf28], k3g[:, 0:qf28],
                                               MAGIC, sc3g[:, 0:qf28],
                                               sub, sub)
                t3g, t3gv = gt("t3g")
                nc.scalar.activation(t3g[:, 0:qf28], f3g[:, 0:qf28], Sin,
                                     bias=0.0, scale=NEG2PI)
                u0g, u0gv = gt("u0g")
                u1g, u1gv = gt("u1g")
                # L2 pairing: l2-run = [o3_0, o3_2], r2-run = [o3_1, o3_3]
                opos = {0: 0, 2: 7, 1: 14, 3: 21}
                for n in range(4):
                    cn = L3[n]
                    sl = (slice(None), slice(0, q), slice(7 * n, 7 * n + 7))
                    nc.scalar.activation(u0gv[sl], p3gv[sl], Ident,
                                         bias=cn["beta"], scale=cn["P"])
                    nc.vector.scalar_tensor_tensor(
                        u1gv[sl], t3gv[sl], cn["R"], u0gv[sl], mult, addop)
                    nc.vector.scalar_tensor_tensor(
                        o3accv[:, 0:q, opos[n]:opos[n] + 7], s3Cgv[sl],
                        cn["A"], u1gv[sl], mult, addop)

                # ---- level 2 + level 1 (batched over the group) ----
                l2 = o3accv[:, 0:q, 0:14]
                r2 = o3accv[:, 0:q, 14:28]
                s2, s2f = gt("s2")
                s2v = s2f[:, 0:q, :]
                nc.gpsimd.tensor_add(s2v, l2, r2)
                p2, p2f = gt("p2")
                p2v = p2f[:, 0:q, :]
                nc.gpsimd.tensor_mul(p2v, l2, r2)
                qf = q * 14
                sc2, _ = gt("sc2")
                nc.vector.tensor_scalar_mul(sc2[:, 0:qf], s2[:, 0:qf], INV2PI)
                k2, _ = gt("k2")
                nc.vector.tensor_scalar_add(k2[:, 0:qf], sc2[:, 0:qf], MAGIC)
                f2, _ = gt("f2")
                nc.vector.scalar_tensor_tensor(f2[:, 0:qf], k2[:, 0:qf], MAGIC,
                                               sc2[:, 0:qf], sub, sub)
                t2, t2f = gt("t2")
                t2v = t2f[:, 0:q, :]
                nc.scalar.activation(t2[:, 0:qf], f2[:, 0:qf], Sin, bias=0.0,
                                     scale=NEG2PI)
                u0_2, u0_2f = gt("u0_2")
                u0_2v = u0_2f[:, 0:q, :]
                u1_2, u1_2f = gt("u1_2")
                u1_2v = u1_2f[:, 0:q, :]
                o2, o2f = gt("o2")
                o2v = o2f[:, 0:q, :]
                for m in range(2):
                    cm = L2[m]
                    sl = (slice(None), slice(0, q), slice(7 * m, 7 * m + 7))
                    nc.scalar.activation(u0_2f[sl], p2f[sl], Ident,
                                         bias=cm["beta"], scale=cm["P"])
                    nc.vector.scalar_tensor_tensor(
                        u1_2f[sl], t2f[sl], cm["R"], u0_2f[sl], mult, addop)
                    nc.vector.scalar_tensor_tensor(
                        o2f[sl], s2f[sl], cm["A"], u1_2f[sl], mult, addop)
                l1 = o2v[:, :, 0:7]
                r1 = o2v[:, :, 7:14]
                qf = q * 7
                s1, s1f = gt("s1")
                s1v = s1f[:, 0:q, :]
                nc.gpsimd.tensor_add(s1v, l1, r1)
                p1, p1f = gt("p1")
                p1v = p1f[:, 0:q, :]
                nc.gpsimd.tensor_mul(p1v, l1, r1)
                sc1, _ = gt("sc1")
                nc.vector.tensor_scalar_mul(sc1[:, 0:qf], s1[:, 0:qf], INV2PI)
                k1, _ = gt("k1")
                nc.vector.tensor_scalar_add(k1[:, 0:qf], sc1[:, 0:qf], MAGIC)
                f1, _ = gt("f1")
                nc.vector.scalar_tensor_tensor(f1[:, 0:qf], k1[:, 0:qf], MAGIC,
                                               sc1[:, 0:qf], sub, sub)
                t1, t1f = gt("t1")
                t1v = t1f[:, 0:q, :]
                nc.scalar.activation(t1[:, 0:qf], f1[:, 0:qf], Sin, bias=0.0,
                                     scale=NEG2PI)
                c1 = L1[0]
                u0_1, u0_1f = gt("u0_1")
                u0_1v = u0_1f[:, 0:q, :]
                nc.scalar.activation(u0_1v, p1v, Ident, bias=c1["beta"],
                                     scale=c1["P"])
                u1_1, u1_1f = gt("u1_1")
                u1_1v = u1_1f[:, 0:q, :]
                nc.vector.scalar_tensor_tensor(
                    u1_1v, t1v, c1["R"], u0_1v, mult, addop)
                yo, yof = gt("yo")
                yov = yof[:, 0:q, :]
                nc.vector.scalar_tensor_tensor(
                    yov, s1v, c1["A"], u1_1v, mult, addop)

                dst = out_d[st0:st0 + glen].transpose([1, 0, 2, 3])
                yo4 = yo[:, 0:qf].rearrange("p (g b a) -> p g b a",
                                            g=glen, a=SLOTS)
                nc.sync.dma_start(dst, yo4)
                st0 += glen

    nc.compile()
    _PROGRAM_CACHE[key] = nc
    return nc


def kernel(x, leaf_w, leaf_b, w1, b1, om1, w2, b2, om2, w3, b3, om3):
    from concourse.bass_interp import get_hw_module
    from concourse.bass_utils import run_bass_kernel_spmd

    L3, L2, L1, wp = _fold(leaf_w, leaf_b, w1, b1, om1, w2, b2, om2, w3, b3, om3)
    _build_program.consts = (L3, L2, L1)
    nc = _build_program()

    in_maps = []
    x = np.ascontiguousarray(x, dtype=F32)
    for c in range(N_CORES):
        xh, xl = _pack_x(x[c * N_CORE:(c + 1) * N_CORE])
        in_maps.append({"xh": xh, "xl": xl, "wp": wp})

    kw = {}
    if os.environ.get("KERNEL_TRACE_DIR"):
        kw["tmpdir"] = os.environ["KERNEL_TRACE_DIR"]
    old = nc.m
    nc.m = get_hw_module(nc.m)
    try:
        res = run_bass_kernel_spmd(nc, in_maps, core_ids=list(range(N_CORES)), **kw)
    finally:
        nc.m = old
    kernel._last = res

    out = np.empty(N_FULL, F32)
    for c in range(N_CORES):
        oc = res.results[c]["out"]          # [N_ST, 128, B, 7]
        oc = oc.transpose(0, 2, 1, 3).reshape(-1)[:N_CORE]
        out[c * N_CORE:(c + 1) * N_CORE] = oc
    return out



# revision 12
# speedup vs baseline: 1.2739x; 1.2739x over previous
"""BinaryTreeRNN Trainium2 kernel — 8-core data-parallel, fp16 pipeline.

Contract: kernel(**inputs) takes FULL unsharded inputs (x [4M,16] f32 plus tiny
tree params) and returns the FULL [4M] f32 output.

Design (per core, N_core = 500k samples, padded to 501760 = 560 blocks x 896):
  * Host folds tree params (float64):  softmax(om) -> per-node (A,P,R,phi,B);
    the combine  o = A*s + R*sin(s+phi) + P*l*r + B  is refactored as
      o = HL*HR + R*sin(theta) + const,   HL = c_hl*(P*l+A), HR = c_hr*(r+A/P)
    (factored quadratic absorbs the linear A*s term).  Stored values carry
    affine maps  true = S*stored + T  folded into the next level's constants;
    per-level power-of-2 scales keep everything in fp16 range.
  * PE: per block one fp16 matmul  out[p,c] = sum_k xt[k,p]*wp[k,c] producing
    12 funcs x 7 slots = 84 cols: HL3/HR3 (4 nodes, L2-pair-permuted) and
    sc3 = (s3+phi3)/2pi.  Bias via two constant rows (112=hi,113=lo).
  * Tree on DVE/ACT/Pool in fp16:  sin via fp16 write-rounding magic
    (k = ts_add(sc,1536) rounds at fp16 write; ts_sub back; f = k - sc;
    ACT Sin(-2pi*f + bias)), products via tt_mul, per-node affines via
    dual-scalar tensor_scalar, R-scaling via broadcast-AP tensor_tensor.
"""

import os
import sys

for _p in ("/opt/trn_rl_repo", "/root/.axon_site/_ro/trn_rl_repo"):
    if os.path.isdir(_p) and _p not in sys.path:
        sys.path.append(_p)

import numpy as np

N_FULL = 4_000_000
V = 16
N_CORES = 8
N_CORE = N_FULL // N_CORES          # 500_000
SLOTS = 7
BLK = 128 * SLOTS                   # 896
N_BLOCKS = 560
N_PAD = N_BLOCKS * BLK              # 501_760
B = 16                              # blocks per supertile
N_ST = N_BLOCKS // B                # 35
GROUP = 5                           # supertiles per group

TWO_PI = 2.0 * np.pi
M16 = 1536.0                        # fp16 round-to-int magic
PERM = [0, 2, 1, 3]                 # L3 node order: L2 pairs contiguous

F16 = np.float16
F32 = np.float32
F64 = np.float64


def _softmax64(om):
    e = np.exp(om.astype(F64) - om.astype(F64).max(-1, keepdims=True))
    return e / e.sum(-1, keepdims=True)


def _lvl(w, b, om):
    sm = _softmax64(om)
    w64 = w.astype(F64)
    A = w64 * sm[:, 0]
    S = w64 * sm[:, 1]
    C = w64 * sm[:, 2]
    P = w64 * sm[:, 3]
    return dict(A=A, B=b.astype(F64), P=P, R=np.hypot(S, C),
                phi=np.arctan2(C, S))


def _pow2(v):
    return float(2.0 ** np.round(np.log2(max(abs(float(v)), 1e-30))))


def _fold(leaf_w, leaf_b, w1, b1, om1, w2, b2, om2, w3, b3, om3, x_sample):
    """float64 constant folding -> (wp fp16 [128,84], consts dict)."""
    L3 = _lvl(w3, b3, om3)
    L2 = _lvl(w2, b2, om2)
    L1 = _lvl(w1, b1, om1)
    lw = leaf_w.astype(F64)
    lb = leaf_b.astype(F64)
    h = (x_sample.astype(F64) @ lw.T + lb).T      # [8, M]

    def calib(vals, target=2.0):
        return _pow2(target / (np.sqrt((vals ** 2).mean()) + 1e-30))

    # ---- L3 ----
    n3 = []
    o3t = []
    for n in range(4):
        A, P, R, phi, Bc = (L3[k][n] for k in ("A", "P", "R", "phi", "B"))
        l, r = h[2 * n], h[2 * n + 1]
        c_hl = calib(P * l + A)
        c_hr = calib(r + A / P)
        o3t.append(A * (l + r) + R * np.sin(l + r + phi) + P * l * r + Bc)
        n3.append(dict(A=A, P=P, R=R, phi=phi, B=Bc, c_hl=c_hl, c_hr=c_hr,
                       wl=lw[2 * n], wr=lw[2 * n + 1], bl=lb[2 * n],
                       br=lb[2 * n + 1]))
    for m in range(2):
        a, b_ = n3[2 * m], n3[2 * m + 1]
        cc = _pow2(np.sqrt(a["c_hl"] * a["c_hr"] * b_["c_hl"] * b_["c_hr"]))
        a["c_hr"] *= cc / (a["c_hl"] * a["c_hr"])
        b_["c_hr"] *= cc / (b_["c_hl"] * b_["c_hr"])
        a["c"] = b_["c"] = cc
        for d in (a, b_):
            d["S"] = 1.0 / cc
            d["T"] = d["B"] - d["A"] ** 2 / d["P"]

    # ---- L2 ----
    n2 = []
    o2t = []
    for m in range(2):
        A, P, R, phi, Bc = (L2[k][m] for k in ("A", "P", "R", "phi", "B"))
        cl, cr = n3[2 * m], n3[2 * m + 1]
        l, r = o3t[2 * m], o3t[2 * m + 1]
        c_hl = calib(P * l + A)
        c_hr = calib(r + A / P)
        o2t.append(A * (l + r) + R * np.sin(l + r + phi) + P * l * r + Bc)
        th_b = cl["T"] + cr["T"] + phi
        n2.append(dict(
            A=A, P=P, R=R, phi=phi, B=Bc, c_hl=c_hl, c_hr=c_hr,
            hl_sc=P * cl["S"] * c_hl, hl_b=(A + P * cl["T"]) * c_hl,
            hr_sc=cr["S"] * c_hr, hr_b=(cr["T"] + A / P) * c_hr,
            sc_sc=cl["S"] / TWO_PI,
            dfrac=(th_b / TWO_PI) - np.round(th_b / TWO_PI),
        ))
    a, b_ = n2
    cc = _pow2(np.sqrt(a["c_hl"] * a["c_hr"] * b_["c_hl"] * b_["c_hr"]))
    for d in (a, b_):
        c0 = d["c_hl"] * d["c_hr"]
        d["c_hr"] *= cc / c0
        d["hr_sc"] *= cc / c0
        d["hr_b"] *= cc / c0
        d["c"] = cc
        d["S"] = 1.0 / cc
        d["T"] = d["B"] - d["A"] ** 2 / d["P"]

    # ---- L1 ----
    A, P, R, phi, Bc = (L1[k][0] for k in ("A", "P", "R", "phi", "B"))
    cl, cr = n2
    l, r = o2t
    c_hl = calib(P * l + A)
    c_hr = calib(r + A / P)
    th_b = cl["T"] + cr["T"] + phi
    n1 = dict(
        A=A, P=P, R=R, phi=phi, B=Bc, c_hl=c_hl, c_hr=c_hr,
        hl_sc=P * cl["S"] * c_hl, hl_b=(A + P * cl["T"]) * c_hl,
        hr_sc=cr["S"] * c_hr, hr_b=(cr["T"] + A / P) * c_hr,
        sc_sc=cl["S"] / TWO_PI,
        dfrac=(th_b / TWO_PI) - np.round(th_b / TWO_PI),
        c=c_hl * c_hr,
    )
    n1["S"] = 1.0 / n1["c"]
    n1["T"] = Bc - A ** 2 / P

    # ---- PE weight matrix [128, 84]: col 7j+a ----
    wp = np.zeros((128, 84), F64)
    for j, n in enumerate(PERM):
        d = n3[n]
        cols = [
            (j, d["wl"] * d["P"] * d["c_hl"],
             (d["P"] * d["bl"] + d["A"]) * d["c_hl"]),
            (4 + j, d["wr"] * d["c_hr"], (d["br"] + d["A"] / d["P"]) * d["c_hr"]),
            (8 + j, (d["wl"] + d["wr"]) / TWO_PI,
             (d["bl"] + d["br"] + d["phi"]) / TWO_PI),
        ]
        for jj, wv, bias in cols:
            for a_ in range(SLOTS):
                wp[16 * a_:16 * a_ + 16, 7 * jj + a_] = wv
                bh = np.float16(bias)
                wp[112, 7 * jj + a_] = bh
                wp[113, 7 * jj + a_] = np.float16(bias - float(bh))
    wp16 = wp.astype(F16)

    # broadcast R tiles (node-permuted where applicable)
    r3bc = np.zeros(28, F16)
    for j, n in enumerate(PERM):
        r3bc[7 * j:7 * j + 7] = np.float16(n3[n]["R"] * n3[n]["c"])
    r2bc = np.zeros(14, F16)
    for m in range(2):
        r2bc[7 * m:7 * m + 7] = np.float16(n2[m]["R"] * n2[m]["c"])
    r1bc = np.full(7, np.float16(n1["R"] * n1["c"]), F16)

    # fp16 magic rounding at L3/L2 requires |theta|/2pi well below 512
    mx3 = max(np.abs(h[2 * n] + h[2 * n + 1] + n3[n]["phi"]).max()
              for n in range(4)) / TWO_PI
    mx2 = max(np.abs(o3t[2 * m] + o3t[2 * m + 1] + n2[m]["phi"]).max()
              for m in range(2)) / TWO_PI
    # 2.5x extrapolation subsample-max -> full-N max; beyond 512 a tail
    # sample gets a bounded sign-flipped sin (negligible in L2 norm),
    # beyond ~2048 sin output explodes -> hard error.
    assert mx3 * 2.5 < 500 and mx2 * 2.5 < 2000, \
        f"fp16 sin-magic range exceeded: sc3 max {mx3:.1f}, sc2 max {mx2:.1f}"

    consts = dict(L2=n2, L1=n1, S=n1["S"], T=n1["T"])
    return wp16, np.broadcast_to(r3bc, (128, 28)).copy(), \
        np.broadcast_to(r2bc, (128, 14)).copy(), \
        np.broadcast_to(r1bc, (128, 7)).copy(), consts


def _pack_x(x_shard, n_st=N_ST, b_blocks=B):
    """[n,16] f32 -> fp16 [n_st, 112, b_blocks*128] stationary rows."""
    npad = n_st * b_blocks * BLK
    xs = np.empty((npad, V), F32)
    xs[:len(x_shard)] = x_shard
    xs[len(x_shard):] = 1.0
    a = xs.reshape(n_st, b_blocks, 128, SLOTS, V)      # [st, b, p, a, v]
    xt = a.transpose(0, 3, 4, 1, 2).reshape(n_st, 112, b_blocks * 128)
    return np.ascontiguousarray(xt, dtype=F16)


_PROGRAM_CACHE = {}


def _build_program(n_st=N_ST, b_blocks=B, group=GROUP):
    import json
    key = (n_st, b_blocks, group,
           json.dumps(_build_program.consts, sort_keys=True, default=str))
    if key in _PROGRAM_CACHE:
        return _PROGRAM_CACHE[key]

    import concourse.bass as bass
    import concourse.tile as tile
    from concourse import bacc, mybir
    from contextlib import ExitStack

    f32 = mybir.dt.float32
    f16 = mybir.dt.float16
    Sin = mybir.ActivationFunctionType.Sin
    Ident = mybir.ActivationFunctionType.Identity
    sub = mybir.AluOpType.subtract
    mult = mybir.AluOpType.mult
    addop = mybir.AluOpType.add

    C = _build_program.consts
    n2, n1 = C["L2"], C["L1"]

    nc = bacc.Bacc("TRN2", target_bir_lowering=False, debug=False,
                   num_devices=N_CORES)
    xh_d = nc.dram_tensor("xh", [n_st, 112, b_blocks * 128], f16,
                          kind="ExternalInput")
    wp_d = nc.dram_tensor("wp", [128, 84], f16, kind="ExternalInput")
    r3_d = nc.dram_tensor("r3", [128, 28], f16, kind="ExternalInput")
    r2_d = nc.dram_tensor("r2", [128, 14], f16, kind="ExternalInput")
    r1_d = nc.dram_tensor("r1", [128, 7], f16, kind="ExternalInput")
    out_d = nc.dram_tensor("out", [n_st, 128, b_blocks, SLOTS], f16,
                           kind="ExternalOutput")

    def reg_const(v):
        v = float(F32(v))
        if (f32, v) not in nc.const_aps.aps:
            t = nc.alloc_sbuf_tensor(
                f"constx-{len(nc.const_aps.aps)}", [128, 1], f32)
            nc.gpsimd.memset(t.ap(), v)
            nc.const_aps.aps[(f32, v)] = t.ap()

    reg_const(0.0)
    for d in (n2[0], n2[1], n1):
        reg_const(TWO_PI * d["dfrac"])
    nc.all_engine_barrier()

    with tile.TileContext(nc) as tc:
        with ExitStack() as ctx:
            const_pool = ctx.enter_context(tc.tile_pool(name="const", bufs=1))
            xpool = ctx.enter_context(tc.tile_pool(name="x", bufs=1))
            ppool = ctx.enter_context(
                tc.tile_pool(name="ps", bufs=2, space=bass.MemorySpace.PSUM))
            gpool = ctx.enter_context(tc.tile_pool(name="g", bufs=2))

            wp = const_pool.tile([128, 84], f16)
            nc.sync.dma_start(wp[:], wp_d[:])
            r3t = const_pool.tile([128, 28], f16)
            nc.sync.dma_start(r3t[:], r3_d[:])
            r2t = const_pool.tile([128, 14], f16)
            nc.sync.dma_start(r2t[:], r2_d[:])
            r1t = const_pool.tile([128, 7], f16)
            nc.sync.dma_start(r1t[:], r1_d[:])

            # manually double-buffered x tiles: const rows set once
            xts = []
            for i in range(2):
                t = xpool.tile([128, b_blocks * 128], f16, name=f"xt{i}",
                               tag=f"xt{i}")
                # start partition must be a multiple of 32; rows 96:112 are
                # rewritten by every x DMA, rows 112:128 stay constant 1.0
                nc.gpsimd.memset(t[96:128, :], 1.0)
                xts.append(t)

            st0 = 0
            while st0 < n_st:
                glen = min(group, n_st - st0)
                q = glen * b_blocks

                def gt(cols, nm, dt=f16):
                    t = gpool.tile([128, group * b_blocks * cols], dt,
                                   name=nm, tag=nm)
                    return t, t[:].rearrange("p (q c) -> p q c", c=cols)

                g3, g3v = gt(28, "g3")
                c3, c3v = gt(28, "c3")
                hl3, hl3v = gt(28, "hl3")

                for seg in range(glen):
                    st = st0 + seg
                    xt = xts[st % 2]
                    nc.sync.dma_start(xt[0:112, :], xh_d[st])
                    ps = ppool.tile([128, b_blocks * 128], f32)
                    for b in range(b_blocks):
                        nc.tensor.matmul(ps[:, 128 * b:128 * b + 84],
                                         xt[:, 128 * b:128 * b + 128],
                                         wp[:], start=True, stop=True)
                    psv = ps[:].rearrange("p (b c) -> p b c", c=128)
                    ssl = slice(seg * b_blocks, (seg + 1) * b_blocks)
                    # evacuate PSUM (only one PSUM operand allowed per op):
                    nc.scalar.activation(hl3v[:, ssl, :], psv[:, :, 0:28],
                                         Ident, bias=0.0, scale=1.0)
                    nc.vector.tensor_tensor(g3v[:, ssl, :], hl3v[:, ssl, :],
                                            psv[:, :, 28:56], mult)
                    nc.scalar.activation(c3v[:, ssl, :], psv[:, :, 56:84],
                                         Ident, bias=0.0, scale=1.0)

                qsl = slice(0, q)
                qf = q * 28

                # ---- L3 sin path (fp16 magic round) ----
                k3, k3v = gt(28, "k3")
                nc.vector.tensor_scalar_add(k3[:, 0:qf], c3[:, 0:qf], M16)
                nc.vector.tensor_scalar_sub(k3[:, 0:qf], k3[:, 0:qf], M16)
                f3, f3v = gt(28, "f3")
                nc.gpsimd.tensor_tensor(f3[:, 0:qf], k3[:, 0:qf],
                                        c3[:, 0:qf], sub)
                t3, t3v = gt(28, "t3")
                nc.scalar.activation(t3[:, 0:qf], f3[:, 0:qf], Sin, bias=0.0,
                                     scale=float(F32(-TWO_PI)))
                u3, u3v = gt(28, "u3")
                r3b = r3t[:].unsqueeze(1).broadcast_to((128, q, 28))
                nc.vector.tensor_tensor(u3v[:, qsl, :], t3v[:, qsl, :],
                                        r3b, mult)
                o3, o3v = gt(28, "o3")
                nc.vector.tensor_tensor(o3[:, 0:qf], g3[:, 0:qf],
                                        u3[:, 0:qf], addop)

                # ---- L2 ----
                l2v = o3v[:, qsl, 0:14]
                r2v_ = o3v[:, qsl, 14:28]
                s2, s2v = gt(14, "s2")
                nc.gpsimd.tensor_tensor(s2v[:, qsl, :], l2v, r2v_, addop)
                sc2, sc2v = gt(14, "sc2")
                k2, k2v = gt(14, "k2")
                HL2, HL2v = gt(14, "HL2")
                HR2, HR2v = gt(14, "HR2")
                t2, t2v = gt(14, "t2")
                f2, f2v = gt(14, "f2")
                for m in range(2):
                    d = n2[m]
                    sl = (slice(None), qsl, slice(7 * m, 7 * m + 7))
                    nc.vector.tensor_scalar_mul(sc2v[sl], s2v[sl],
                                                float(F32(d["sc_sc"])))
                    nc.vector.tensor_scalar_add(
                        k2v[sl], sc2v[sl], float(F32(M16 + d["dfrac"])))
                    nc.vector.tensor_scalar_sub(k2v[sl], k2v[sl], M16)
                    nc.vector.tensor_scalar(
                        HL2v[sl], l2v[:, :, 7 * m:7 * m + 7],
                        float(F32(d["hl_sc"])), float(F32(d["hl_b"])),
                        op0=mult, op1=addop)
                    nc.vector.tensor_scalar(
                        HR2v[sl], r2v_[:, :, 7 * m:7 * m + 7],
                        float(F32(d["hr_sc"])), float(F32(d["hr_b"])),
                        op0=mult, op1=addop)
                nc.vector.tensor_tensor(f2[:, 0:q * 14], k2[:, 0:q * 14],
                                        sc2[:, 0:q * 14], sub)
                for m in range(2):
                    d = n2[m]
                    sl = (slice(None), qsl, slice(7 * m, 7 * m + 7))
                    nc.scalar.activation(t2v[sl], f2v[sl], Sin,
                                         bias=float(F32(TWO_PI * d["dfrac"])),
                                         scale=float(F32(-TWO_PI)))
                g2, g2v = gt(14, "g2")
                nc.vector.tensor_tensor(g2[:, 0:q * 14], HL2[:, 0:q * 14],
                                        HR2[:, 0:q * 14], mult)
                u2, u2v = gt(14, "u2")
                r2b = r2t[:].unsqueeze(1).broadcast_to((128, q, 14))
                nc.vector.tensor_tensor(u2v[:, qsl, :], t2v[:, qsl, :],
                                        r2b, mult)
                o2, o2v = gt(14, "o2")
                nc.vector.tensor_tensor(o2[:, 0:q * 14], g2[:, 0:q * 14],
                                        u2[:, 0:q * 14], addop)

                # ---- L1 (fp32 magic: phases exceed the fp16 +/-512 range) ----
                l1v = o2v[:, qsl, 0:7]
                r1v_ = o2v[:, qsl, 7:14]
                s1, s1v = gt(7, "s1")
                nc.gpsimd.tensor_tensor(s1v[:, qsl, :], l1v, r1v_, addop)
                d = n1
                M32 = float(1.5 * 2.0 ** 23)
                sc1, sc1v = gt(7, "sc1", dt=f32)
                nc.vector.tensor_scalar_mul(sc1[:, 0:q * 7], s1[:, 0:q * 7],
                                            float(F32(d["sc_sc"])))
                k1, k1v = gt(7, "k1", dt=f32)
                nc.vector.tensor_scalar(k1[:, 0:q * 7], sc1[:, 0:q * 7],
                                        float(F32(d["dfrac"])), M32,
                                        op0=addop, op1=addop)
                nc.vector.tensor_scalar(k1[:, 0:q * 7], k1[:, 0:q * 7],
                                        M32, float(F32(d["dfrac"])),
                                        op0=sub, op1=sub)
                f1, f1v = gt(7, "f1")
                nc.vector.tensor_tensor(f1[:, 0:q * 7], k1[:, 0:q * 7],
                                        sc1[:, 0:q * 7], sub)
                t1, t1v = gt(7, "t1")
                nc.scalar.activation(t1[:, 0:q * 7], f1[:, 0:q * 7], Sin,
                                     bias=0.0, scale=float(F32(-TWO_PI)))
                HL1, HL1v = gt(7, "HL1")
                nc.vector.tensor_scalar(
                    HL1v[:, qsl, :], l1v, float(F32(d["hl_sc"])),
                    float(F32(d["hl_b"])), op0=mult, op1=addop)
                HR1, HR1v = gt(7, "HR1")
                nc.vector.tensor_scalar(
                    HR1v[:, qsl, :], r1v_, float(F32(d["hr_sc"])),
                    float(F32(d["hr_b"])), op0=mult, op1=addop)
                g1, g1v = gt(7, "g1")
                nc.vector.tensor_tensor(g1[:, 0:q * 7], HL1[:, 0:q * 7],
                                        HR1[:, 0:q * 7], mult)
                u1, u1v = gt(7, "u1")
                r1b = r1t[:].unsqueeze(1).broadcast_to((128, q, 7))
                nc.vector.tensor_tensor(u1v[:, qsl, :], t1v[:, qsl, :],
                                        r1b, mult)
                yo, yov = gt(7, "yo")
                nc.vector.tensor_tensor(yo[:, 0:q * 7], g1[:, 0:q * 7],
                                        u1[:, 0:q * 7], addop)

                dst = out_d[st0:st0 + glen].transpose([1, 0, 2, 3])
                yo4 = yo[:, 0:q * 7].rearrange("p (g b a) -> p g b a",
                                               g=glen, a=SLOTS)
                nc.sync.dma_start(dst, yo4)
                st0 += glen

    nc.compile()
    _PROGRAM_CACHE[key] = nc
    return nc


def kernel(x, leaf_w, leaf_b, w1, b1, om1, w2, b2, om2, w3, b3, om3):
    from concourse.bass_interp import get_hw_module
    from concourse.bass_utils import run_bass_kernel_spmd

    x = np.ascontiguousarray(x, dtype=F32)
    wp, r3, r2, r1, consts = _fold(
        leaf_w, leaf_b, w1, b1, om1, w2, b2, om2, w3, b3, om3,
        x[:: max(1, N_FULL // 4096)][:4096])
    _build_program.consts = consts
    nc = _build_program()

    in_maps = []
    for c in range(N_CORES):
        xh = _pack_x(x[c * N_CORE:(c + 1) * N_CORE])
        in_maps.append({"xh": xh, "wp": wp, "r3": r3, "r2": r2, "r1": r1})

    kw = {}
    if os.environ.get("KERNEL_TRACE_DIR"):
        kw["tmpdir"] = os.environ["KERNEL_TRACE_DIR"]
    old = nc.m
    nc.m = get_hw_module(nc.m)
    try:
        res = run_bass_kernel_spmd(nc, in_maps, core_ids=list(range(N_CORES)),
                                   **kw)
    finally:
        nc.m = old
    kernel._last = res

    S, T = consts["S"], consts["T"]
    out = np.empty(N_FULL, F32)
    for c in range(N_CORES):
        oc = res.results[c]["out"]          # [N_ST, 128, B, 7] f16
        oc = oc.transpose(0, 2, 1, 3).reshape(-1)[:N_CORE].astype(F32)
        out[c * N_CORE:(c + 1) * N_CORE] = F32(S) * oc + F32(T)
    return out
